# revision 21
# baseline (speedup 1.0000x reference)
"""Trainium2 Bass kernel for nn_LinearLoopLayer: out = x @ weight.T + bias.

x: (2048, 4096) f32, weight: (4096, 4096) f32, bias: (4096,) f32.
Sharding: 2 batch-halves x 4 out-feature-quarters across 8 NeuronCores.
Each core computes outT_shard[j, b] = sum_i wT[i, j] * xT[i, b] + bias[j]
as 512 bf16 matmuls (N=512, warm floor 109.2us @ 2.4GHz).

Trace-driven design (baseline 135.6us -> ~128us). Measured constraints:
  - exec_time spans first REGULAR instruction (~6us into the NEFF) to the
    end of a fixed ~6.1us framework semaphore-reset teardown; preamble
    barriers before and that teardown are not controllable from Bass IR.
  - The early load phase (7-16us) is latency-bound while all 8 cores'
    rings fetch at once: ~60-130GB/s per ring, ~200-300GB/s per core,
    regardless of ring count. Pass-A consumption is an invariant 384KB
    per 1.7us (225GB/s) for any 8-psum-bank tiling, so the stream start
    and early chunk deadlines decide everything.
  - The HAM clock gate needs ~3.4us of gap-free PE activity to reach
    2.4GHz and re-throttles after a ~3.4us idle; a single late chunk that
    stalls the PE >3.4us costs 2x via the cold clock (the baseline's
    failure mode).
Design:
  - Tile-major DRAM layout ([KT, 128, cols] - one contiguous block per
    k-tile) so early pieces read adjacent contiguous runs instead of
    64KB-strided 1-2KB segments; measured ~15-20% faster early supply.
  - Per-k-tile pieces (xt_k 256KB / wta_k 128KB) rotate across all three
    rings (sync + scalar HWDGE, gpsimd SWDGE) in consumption order: each
    ring owes exactly 384KB per 3 k-tiles (75GB/s), always feasible, so
    stalls stay well under the HAM window. wtb/bias FIFO-trail behind;
    stores ride gpsimd (last two on the by-then-idle HWDGE rings, whose
    completion receipt is ~0.6us vs ~2us for SWDGE).
  - 33 warmup matmuls bridge PE activity from the preamble to the first
    data-gated matmul (~10.5us), so the clock is warm at stream start and
    the 109.2us matmul stream runs gapless and warm end-to-end.
  - Tail: pass C (jb7) runs bank-sequential (bb0 512 cols, then 2x256) so
    only a 256-col drain + 64KB store + receipt (~2.4us) stays exposed
    before the fixed teardown.
  - bf16 inputs/outputs: rel err ~3.1e-3 vs the 2e-2 gate (fp8 cannot
    pass the gate even single-operand; int8 matmul unsupported).
"""

import sys

import numpy as np

sys.path.insert(0, "/opt/trn_rl_repo")

import concourse.mybir as mybir
from concourse import bacc, tile
from concourse.bass_utils import run_bass_kernel_spmd

P = 128
B, K, J = 2048, 4096, 4096
NCORES = 8
B_SPLIT, J_SPLIT = 2, 4
BL, JL = B // B_SPLIT, J // J_SPLIT  # per-core local batch / out-features
KT = K // P  # contraction tiles (32)
JB = JL // P  # 128-feature j-blocks per core (8)
NB = BL // 512  # 512-col batch blocks per core (2)
JH = JL // 2  # out-feature half (512) - wta/wtb split

# it-blocks for the interleaved load schedule: fine early (1 k-tile) so the
# first matmuls gate on tiny transfers, coarse late to amortize the ~0.6us
# per-trigger cost on the issuing queue. Blocks rotate across THREE rings
# (sync/scalar HWDGE + gpsimd SWDGE): the early phase is latency-bound at
# ~60-130GB/s per ring while all 8 cores' rings are in flight, so per-ring
# need must stay ~75GB/s. Within a ring the FIFO order equals consumption
# order.
WARMUP_MMS = 33  # see load schedule comment below


PASS_A = (0, 1, 2, 3)  # 8 psum banks (x NB)
PASS_B = (4, 5, 6)     # 6 banks
# pass C (jb 7) is emitted bank-sequential with bb1 split into 2x256 cols

_NP_BF16 = mybir.dt.np(mybir.dt.bfloat16)


def _build():
    nc = bacc.Bacc(None, target_bir_lowering=False)
    bf16 = mybir.dt.bfloat16
    f32 = mybir.dt.float32
    # tile-major DRAM layout: one [P, cols] tile per k-tile, so each load
    # piece reads adjacent-partition contiguous runs instead of 64KB-strided
    # 1-2KB segments (DRAM page locality for the latency-bound early phase)
    xt = nc.declare_dram_parameter("xt", [KT, P, BL], bf16, isOutput=False)
    wta = nc.declare_dram_parameter("wta", [KT, P, JH], bf16, isOutput=False)
    wtb = nc.declare_dram_parameter("wtb", [KT, P, JH], bf16, isOutput=False)
    biasT = nc.declare_dram_parameter("biasT", [P, JB], f32, isOutput=False)
    # out stored as bf16 (host upcasts): halves store DMA and the exposed
    # final-store tail; adds ~1e-3 rel err on top of the input quantization
    out = nc.declare_dram_parameter("out", [JL, BL], bf16, isOutput=True)

    with tile.TileContext(nc) as tc:
        with (
            tc.tile_pool(name="xp", bufs=1) as xp,
            tc.tile_pool(name="wap", bufs=1) as wap,
            tc.tile_pool(name="wbp", bufs=1) as wbp,
            tc.tile_pool(name="biasp", bufs=1) as biasp,
            tc.tile_pool(name="outp", bufs=4) as outp,
            tc.tile_pool(name="psum", bufs=8, space="PSUM") as psum_pool,
        ):
            xt_sb = xp.tile([P, KT * BL], bf16)
            wta_sb = wap.tile([P, KT * JH], bf16)
            wtb_sb = wbp.tile([P, KT * JH], bf16)
            bias_sb = biasp.tile([P, JB], f32)

            # HAM warm-up: a few dummy matmuls start PE activity during the
            # preamble window so the clock gate flips to 2.4GHz ~3.4us after
            # first activity - right around when the first data-gated
            # matmuls begin.
            warm_sb = outp.tile([P, P], bf16, name="warm")
            nc.vector.memset(warm_sb[:], 0)
            warm_ps = psum_pool.tile([P, 512], f32, name="ps")
            for _ in range(WARMUP_MMS):
                nc.tensor.matmul(
                    warm_ps[:, :P], warm_sb[:], warm_sb[:],
                    start=True, stop=True,
                )

            # --- load schedule -------------------------------------------
            # Per-k-tile pieces (xt_k 256KB / wta_k 128KB) rotate across the
            # three rings in deadline order: each ring carries 384KB per
            # 3-k-tile cycle (75GB/s uniform need vs the ~70-130GB/s
            # per-ring rate observed while all 8 cores' rings are in
            # flight). The warmup matmuls bridge PE activity from ~7.5us to
            # ~11us so the HAM clock gate (needs ~3.4us of gap-free PE
            # activity) flips to 2.4GHz right as the data-gated stream
            # starts.
            rings = [nc.sync, nc.scalar, nc.gpsimd]

            def load(ring, sb, dram, it, c0, c1, cols):
                ring.dma_start(
                    sb[:, it * cols + c0 : it * cols + c1],
                    dram[it, :, c0:c1],
                )

            # k-tile 0: the very first matmul (jb0, bb0) gates on wta cols
            # 0:128 (32KB) + xt cols 0:512 on sync.
            load(nc.sync, wta_sb, wta, 0, 0, P, JH)
            load(nc.sync, xt_sb, xt, 0, 0, 512, BL)
            load(nc.scalar, wta_sb, wta, 0, P, JH, JH)
            load(nc.gpsimd, xt_sb, xt, 0, 512, BL, BL)
            for it in range(1, KT):
                # it2's xt would be the gpsimd (SWDGE, slowest) ring's 3rd
                # piece and lands ~0.7us past its deadline (the one
                # recurring stream stall); route it to sync instead, which
                # has slack there.
                ring_x = nc.sync if it == 2 else rings[it % 3]
                load(ring_x, xt_sb, xt, it, 0, BL, BL)
                load(rings[(it + 1) % 3], wta_sb, wta, it, 0, JH, JH)
            # wtb (4.2MB, first consumed ~63us) FIFO-trails the pass-A
            # stream; bias (tiny, consumed at the first drain ~66us) last.
            for it in range(KT):
                load(rings[(it + 2) % 3], wtb_sb, wtb, it, 0, JH, JH)
            nc.gpsimd.dma_start(bias_sb[:], biasT[:, :])

            # --- compute -------------------------------------------------
            def drain_store(o_cols, ps_tile, jb, bcol0, drain_eng, st_eng):
                o = outp.tile([P, o_cols], bf16, name="o")
                if drain_eng is nc.vector:
                    nc.vector.tensor_scalar_add(
                        o[:], ps_tile[:], bias_sb[:, jb : jb + 1]
                    )
                else:
                    nc.scalar.activation(
                        o[:],
                        ps_tile[:],
                        mybir.ActivationFunctionType.Identity,
                        bias=bias_sb[:, jb : jb + 1],
                    )
                st_eng.dma_start(
                    out[jb * P : (jb + 1) * P, bcol0 : bcol0 + o_cols], o[:]
                )

            for pass_jbs in (PASS_A, PASS_B):
                ps = {
                    (jb, bb): psum_pool.tile([P, 512], f32, name="ps")
                    for jb in pass_jbs
                    for bb in range(NB)
                }
                for it in range(KT):
                    for jb in pass_jbs:
                        wsrc = wta_sb if jb < 4 else wtb_sb
                        jo = it * JH + (jb % 4) * P
                        for bb in range(NB):
                            nc.tensor.matmul(
                                ps[(jb, bb)][:],
                                wsrc[:, jo : jo + P],
                                xt_sb[:, it * BL + bb * 512 : it * BL + (bb + 1) * 512],
                                start=(it == 0),
                                stop=(it == KT - 1),
                            )
                for k, (jb, bb) in enumerate(
                    [(j, b) for j in pass_jbs for b in range(NB)]
                ):
                    drain_store(
                        512, ps[(jb, bb)], jb, bb * 512,
                        nc.vector if k % 2 == 0 else nc.scalar,
                        nc.gpsimd,
                    )

            # pass C (jb 7): bank-sequential so each unit's drain+store hide
            # under the next unit's compute; bb1 split into 2x256 cols so
            # the exposed tail is one 256-col drain + 64KB store.
            jb = 7
            ps_b0 = psum_pool.tile([P, 512], f32, name="ps")
            ps_b1a = psum_pool.tile([P, 256], f32, name="ps")
            ps_b1b = psum_pool.tile([P, 256], f32, name="ps")
            for it in range(KT):
                jo = it * JH + (jb % 4) * P
                nc.tensor.matmul(
                    ps_b0[:], wtb_sb[:, jo : jo + P],
                    xt_sb[:, it * BL : it * BL + 512],
                    start=(it == 0), stop=(it == KT - 1),
                )
            drain_store(512, ps_b0, jb, 0, nc.scalar, nc.gpsimd)
            for it in range(KT):
                jo = it * JH + (jb % 4) * P
                nc.tensor.matmul(
                    ps_b1a[:], wtb_sb[:, jo : jo + P],
                    xt_sb[:, it * BL + 512 : it * BL + 768],
                    start=(it == 0), stop=(it == KT - 1),
                )
            drain_store(256, ps_b1a, jb, 512, nc.vector, nc.sync)
            for it in range(KT):
                jo = it * JH + (jb % 4) * P
                nc.tensor.matmul(
                    ps_b1b[:], wtb_sb[:, jo : jo + P],
                    xt_sb[:, it * BL + 768 : it * BL + 1024],
                    start=(it == 0), stop=(it == KT - 1),
                )
            drain_store(256, ps_b1b, jb, 768, nc.vector, nc.sync)
    nc.finalize()
    return nc


_NC_CACHE = {}


def _get_nc():
    if "bf16" not in _NC_CACHE:
        _NC_CACHE["bf16"] = _build()
    return _NC_CACHE["bf16"]


def _part_major(a2d, cols):
    """[K, cols] f32 -> [KT, P, cols] bf16 (tile-major, contiguous tiles)."""
    return np.ascontiguousarray(a2d.reshape(KT, P, cols)).astype(_NP_BF16)


def _make_in_maps(x, weight, bias):
    x = np.asarray(x, dtype=np.float32)
    if x.ndim == 4:
        x = x.reshape(x.shape[0], -1)
    weight = np.asarray(weight, dtype=np.float32)
    bias = np.asarray(bias, dtype=np.float32)
    in_maps = []
    for c in range(NCORES):
        bh, jq = divmod(c, J_SPLIT)
        xT = x[bh * BL : (bh + 1) * BL].T  # [K, BL]
        wT = weight[jq * JL : (jq + 1) * JL].T  # [K, JL]
        bq = bias[jq * JL : (jq + 1) * JL]
        in_maps.append(
            {
                "xt": _part_major(xT, BL),
                "wta": _part_major(wT[:, :JH], JH),
                "wtb": _part_major(wT[:, JH:], JH),
                "biasT": np.ascontiguousarray(bq.reshape(JB, P).T),
            }
        )
    return in_maps


def _assemble(results):
    out = np.empty((B, J), dtype=np.float32)
    for c in range(NCORES):
        bh, jq = divmod(c, J_SPLIT)
        out[bh * BL : (bh + 1) * BL, jq * JL : (jq + 1) * JL] = (
            results[c]["out"].astype(np.float32).T
        )
    return out


def run(x, weight, bias, mm_dt_name=None, trace=False, **kwargs):
    nc = _get_nc()
    in_maps = _make_in_maps(x, weight, bias)
    res = run_bass_kernel_spmd(
        nc, in_maps, core_ids=list(range(NCORES)), trace=trace, **kwargs
    )
    return _assemble(res.results), res


def kernel(x, weight, bias):
    out, _ = run(x, weight, bias)
    return out
